# revision 14
# baseline (speedup 1.0000x reference)
"""DWDN forward (Wiener deconvolution) Trainium2 Bass kernel.

Sharding: 8 cores = 4 batches x 2 channel-halves. Core k handles batch
b=k//2, reduced-channel half h=k%2 (32 of the 64 reduced channels).
Per core:
  P0: OTF via small DFT matmuls (otf = E @ kern @ E^T), D = |otf|^2
  P1: 1x1 reduce conv (w_reduce slice @ x[b]) -> wf, staged to DRAM in
      image layout (f32r) + an fp16 row-bordered copy for the median path
  P2: per channel image: 3x3-median NSR stats (fp16 min/max network with
      border-band decomposition) + Wiener deconvolution as pad-folded DFT
      matmuls with Hermitian symmetry (freq rows k=0..147 of 294)
  P3: expand-conv partial: pout = w_expand[:, half] @ clear
Host sums the two partials per batch and adds b_expand.

All matmuls in float32r (13-bit mantissa, full PE rate). No ACT LUT
activations and no custom-DVE ops (both crash on this deployment);
division via nc.vector.reciprocal only.
"""

import os
import sys
from contextlib import ExitStack

import numpy as np

for _p in ("/opt/trn_rl_repo", "/root/.axon_site/_ro/trn_rl_repo"):
    if os.path.isdir(_p) and _p not in sys.path:
        sys.path.insert(0, _p)

import concourse.tile as tile  # noqa: E402
from concourse import bacc, mybir  # noqa: E402
from concourse.bass_utils import run_bass_kernel_spmd  # noqa: E402

F32 = mybir.dt.float32
F32R = mybir.dt.float32r
F16 = mybir.dt.float16
AT = mybir.AluOpType
AX = mybir.AxisListType

NF, KS, B, H, W = 256, 19, 4, 256, 256
N = H + 2 * KS          # 294
C = 64                  # reduced channels
CH = 32                 # channels per core
KH = N // 2 + 1         # 148 (Hermitian half)
NPIX = H * W
NTOT = N * N
LT = [128, 128, 38]     # partition-tile sizes for 294
MT = [128, 20]          # partition-tile sizes for 148


def _round_f32r(x):
    b = np.ascontiguousarray(x, dtype=np.float32).view(np.uint32).astype(np.uint64)
    b = (b + 0x200) & np.uint64(0xFFFFFC00)
    b = np.minimum(b, 0xFFFFFFFF).astype(np.uint32)
    return b.view(np.float32)


def _host_constants():
    k = np.arange(N)
    Fm = np.exp(-2j * np.pi * np.outer(k, k) / N)
    Sx = np.zeros((N, H))
    for i in range(N):
        Sx[i, min(max(i - KS, 0), H - 1)] = 1.0
    A = Fm @ Sx                                   # [294, 256]
    At = A.T                                      # [256, 294]
    B1 = np.conj(Fm)[KS:KS + H, :] / N            # [256, 294]
    wk = np.ones(KH)
    wk[1:KH - 1] = 2.0
    B4 = B1[:, :KH] * wk[None, :]                 # [256, 148]
    E = np.exp(-2j * np.pi * np.outer(k, (np.arange(KS) - KS // 2)) / N)  # [294,19]
    f = lambda v: _round_f32r(np.ascontiguousarray(v, np.float32))
    cst = {
        "AtR": f(At.real), "AtI": f(At.imag), "AtIn": f(-At.imag),
        "B1tR": f(B1.T.real), "B1tI": f(B1.T.imag), "B1tIn": f(-B1.T.imag),
        "B4R": f(B4.T.real), "B4In": f(-B4.T.imag),
        "ER": f(E.T.real), "EI": f(E.T.imag), "EIn": f(-E.T.imag),
        "onesr": f(np.ones((1, 128), np.float32)),
        "onesc": f(np.ones((128, 1), np.float32)),
    }
    u = np.ones((H, 1), np.float32)
    u[0, 0] = 20.0
    u[-1, 0] = 20.0
    cst["u16"] = u.astype(np.float16)
    return cst


_CACHE = {}


def _build():
    nc = bacc.Bacc(None, target_bir_lowering=False)
    P = {}
    P["x"] = nc.declare_dram_parameter("x", [NF, NPIX], F32R, isOutput=False)
    P["wrT"] = nc.declare_dram_parameter("wrT", [NF, CH], F32R, isOutput=False)
    P["weT"] = nc.declare_dram_parameter("weT", [CH, NF], F32R, isOutput=False)
    P["kern"] = nc.declare_dram_parameter("kern", [KS, KS], F32R, isOutput=False)
    P["brp"] = nc.declare_dram_parameter("brp", [128, 1], F32, isOutput=False)
    for nm, shp in [("AtR", [NF, N]), ("AtI", [NF, N]), ("AtIn", [NF, N]),
                    ("B1tR", [N, H]), ("B1tI", [N, H]), ("B1tIn", [N, H]),
                    ("B4R", [KH, H]), ("B4In", [KH, H]),
                    ("ER", [KS, N]), ("EI", [KS, N]), ("EIn", [KS, N]),
                    ("onesc", [128, 1])]:
        P[nm] = nc.declare_dram_parameter(nm, shp, F32R, isOutput=False)
    P["onesr"] = nc.declare_dram_parameter("onesr", [1, 128], F32, isOutput=False)
    P["u16"] = nc.declare_dram_parameter("u16", [H, 1], F16, isOutput=False)
    pout_d = nc.declare_dram_parameter("pout", [NF, NPIX], F32, isOutput=True)
    nc._dbg_wf = nc.declare_dram_parameter("dbg_wf", [CH, NPIX], F32, isOutput=True)
    nc._dbg_clear = nc.declare_dram_parameter("dbg_clear", [CH, NPIX], F32, isOutput=True)
    nc._dbg_nsr = nc.declare_dram_parameter("dbg_nsr", [CH, 1], F32, isOutput=True)

    wf_d = nc.dram_tensor("wf_stage", [CH, NPIX], F32R)
    wf16_d = nc.dram_tensor("wf16_stage", [CH, H + 2, W], F16)
    clear_d = nc.dram_tensor("clear_stage", [CH, NPIX], F32R)

    with tile.TileContext(nc) as tc:
        _emit(nc, tc, P, pout_d, wf_d, wf16_d, clear_d)
    nc.finalize()
    return nc


def _load_consts(nc, tc, ctx, P):
    cpool = ctx.enter_context(tc.tile_pool(name="consts", bufs=1))
    CT = {}
    for nm in ("AtR", "AtI", "AtIn"):
        t = cpool.tile([128, 2 * N], F32R, tag=nm)
        for i in range(2):
            nc.sync.dma_start(t[:, i * N:(i + 1) * N], P[nm][i * 128:(i + 1) * 128, :])
        CT[nm] = t
    for nm in ("B1tR", "B1tI", "B1tIn"):
        t = cpool.tile([128, 3 * H], F32R, tag=nm)
        for i in range(3):
            rs = LT[i]
            nc.sync.dma_start(t[0:rs, i * H:i * H + H], P[nm][i * 128:i * 128 + rs, :])
        CT[nm] = t
    for nm in ("B4R", "B4In"):
        t = cpool.tile([128, 2 * H], F32R, tag=nm)
        for i in range(2):
            rs = MT[i]
            nc.sync.dma_start(t[0:rs, i * H:i * H + H], P[nm][i * 128:i * 128 + rs, :])
        CT[nm] = t
    for nm in ("ER", "EI", "EIn"):
        t = cpool.tile([KS, N], F32R, tag=nm)
        nc.sync.dma_start(t[:], P[nm][:, :])
        CT[nm] = t
    for nm, shp, dt in [("onesr", [1, 128], F32), ("onesc", [128, 1], F32R)]:
        t = cpool.tile(shp, dt, tag=nm)
        nc.sync.dma_start(t[:], P[nm][:, :])
        CT[nm] = t
    u16 = cpool.tile([128, 2], F16, tag="u16")
    for i in range(2):
        nc.sync.dma_start(u16[:, i:i + 1], P["u16"][i * 128:(i + 1) * 128, :])
    CT["u16"] = u16
    kern_sb = cpool.tile([KS, KS], F32R, tag="kern")
    nc.sync.dma_start(kern_sb[:], P["kern"][:, :])
    CT["kern"] = kern_sb
    wrT = cpool.tile([128, 2 * CH], F32R, tag="wrT")
    for i in range(2):
        nc.sync.dma_start(wrT[:, i * CH:(i + 1) * CH], P["wrT"][i * 128:(i + 1) * 128, :])
    CT["wrT"] = wrT
    weT = cpool.tile([CH, NF], F32R, tag="weT")
    nc.sync.dma_start(weT[:], P["weT"][:, :])
    CT["weT"] = weT
    brp = cpool.tile([128, 1], F32, tag="brp")
    nc.sync.dma_start(brp[:], P["brp"][:, :])
    CT["brp"] = brp
    # persistent P0 outputs
    CT["otfR"] = cpool.tile([128, 3 * KH], F32, name="otfR", tag="otfR")
    CT["otfI"] = cpool.tile([128, 3 * KH], F32, name="otfI", tag="otfI")
    CT["Dt"] = cpool.tile([128, 3 * KH], F32, name="Dtt", tag="Dtt")
    return CT


def _emit(nc, tc, P, pout_d, wf_d, wf16_d, clear_d):
    with ExitStack() as ctx:
        CT = _load_consts(nc, tc, ctx, P)

        # ---- P0 + P1 (scoped pools) ----
        with tc.tile_pool(name="p01", bufs=3) as wp, \
             tc.tile_pool(name="p01ps", bufs=1, space="PSUM") as pp:
            _p0_otf(nc, wp, pp, CT)
            _p1_reduce(nc, wp, pp, CT, P, wf_d, wf16_d)

        nc.sync.dma_start(nc._dbg_wf[:, :], wf_d[:, :].bitcast(F32))
        tc.strict_bb_all_engine_barrier()

        # ---- P2 ----
        with tc.tile_pool(name="sw", bufs=2) as spool, \
             tc.tile_pool(name="med", bufs=2) as mpool, \
             tc.tile_pool(name="p2ps", bufs=1, space="PSUM") as ppool:
            for ci in range(CH):
                _image(nc, spool, mpool, ppool, CT, wf_d, wf16_d, clear_d, ci)

        nc.sync.dma_start(nc._dbg_clear[:, :], clear_d[:, :].bitcast(F32))
        tc.strict_bb_all_engine_barrier()

        # ---- P3 ----
        with tc.tile_pool(name="p3", bufs=4) as wp, \
             tc.tile_pool(name="p3ps", bufs=2, space="PSUM") as pp:
            clr_view = clear_d.rearrange("c (J s) -> J c s", s=512)
            for j in range(NPIX // 512):
                ct = wp.tile([CH, 512], F32R, tag="cchunk")
                nc.sync.dma_start(ct[:], clr_view[j])
                ps3 = pp.tile([128, 512], F32, tag="ps_p3a")
                ps3b = pp.tile([128, 512], F32, tag="ps_p3b")
                nc.tensor.matmul(ps3[:], CT["weT"][:, 0:128], ct[:], start=True, stop=True)
                nc.tensor.matmul(ps3b[:], CT["weT"][:, 128:256], ct[:], start=True, stop=True)
                ot = wp.tile([128, 1024], F32, tag="p3out")
                nc.vector.tensor_copy(ot[:, 0:512], ps3[:])
                nc.scalar.copy(ot[:, 512:1024], ps3b[:])
                nc.sync.dma_start(pout_d[0:128, j * 512:(j + 1) * 512], ot[:, 0:512])
                nc.sync.dma_start(pout_d[128:256, j * 512:(j + 1) * 512], ot[:, 512:1024])


def _p0_otf(nc, wp, pp, CT):
    # Tt[j,k] = sum_i kern[i,j] E[k,i]: lhsT=kern [19,19], rhs=ER/EI [19,294]
    ps_a = pp.tile([KS, N], F32, tag="ps_otfa")
    ps_b = pp.tile([KS, N], F32, tag="ps_otfb")
    nc.tensor.matmul(ps_a[:], CT["kern"][:], CT["ER"][:], start=True, stop=True)
    nc.tensor.matmul(ps_b[:], CT["kern"][:], CT["EI"][:], start=True, stop=True)
    TtR = wp.tile([KS, N], F32R, tag="TtR")
    TtI = wp.tile([KS, N], F32R, tag="TtI")
    nc.vector.tensor_copy(TtR[:], ps_a[:])
    nc.vector.tensor_copy(TtI[:], ps_b[:])
    # otf_t[l,k] = sum_j E[l,j] Tt[j,k]: lhsT = E^T-as-[j,l] = ER/EI slices
    for lt in range(3):
        rs = LT[lt]
        lsl = slice(lt * 128, lt * 128 + rs)
        osl = slice(lt * KH, lt * KH + KH)
        ps_r = pp.tile([128, KH], F32, tag="ps_otfr")
        ps_i = pp.tile([128, KH], F32, tag="ps_otfi")
        nc.tensor.matmul(ps_r[0:rs, :], CT["ER"][:, lsl], TtR[:, 0:KH], start=True, stop=False)
        nc.tensor.matmul(ps_r[0:rs, :], CT["EIn"][:, lsl], TtI[:, 0:KH], start=False, stop=True)
        nc.tensor.matmul(ps_i[0:rs, :], CT["ER"][:, lsl], TtI[:, 0:KH], start=True, stop=False)
        nc.tensor.matmul(ps_i[0:rs, :], CT["EI"][:, lsl], TtR[:, 0:KH], start=False, stop=True)
        nc.vector.tensor_copy(CT["otfR"][0:rs, osl], ps_r[0:rs, :])
        nc.vector.tensor_copy(CT["otfI"][0:rs, osl], ps_i[0:rs, :])
        t2 = wp.tile([128, KH], F32, tag="d_tmp")
        nc.vector.tensor_mul(CT["Dt"][0:rs, osl], CT["otfR"][0:rs, osl], CT["otfR"][0:rs, osl])
        nc.vector.tensor_mul(t2[0:rs, :], CT["otfI"][0:rs, osl], CT["otfI"][0:rs, osl])
        nc.vector.tensor_add(CT["Dt"][0:rs, osl], CT["Dt"][0:rs, osl], t2[0:rs, :])


def _p1_reduce(nc, wp, pp, CT, P, wf_d, wf16_d):
    for j0 in range(NPIX // 2048):  # 32 groups of 4 chunks x 512 px
        xt = [wp.tile([128, 2048], F32R, tag=f"xchunk{i}", name=f"xchunk{i}") for i in range(2)]
        for i in range(2):
            nc.sync.dma_start(xt[i][:], P["x"][i * 128:(i + 1) * 128,
                                               j0 * 2048:(j0 + 1) * 2048])
        psg = [pp.tile([32, 512], F32, tag=f"ps_p1{g}", name=f"psp1{g}")
               for g in range(4)]
        for g in range(4):
            for kt in range(2):
                nc.tensor.matmul(
                    psg[g][:],
                    CT["wrT"][:, kt * CH:(kt + 1) * CH],
                    xt[kt][:, g * 512:(g + 1) * 512],
                    start=(kt == 0), stop=(kt == 1),
                )
        for g in range(4):
            jj = 4 * j0 + g
            wfg = wp.tile([32, 512], F32R, tag=f"wfg{g}", name=f"wfg{g}")
            nc.vector.tensor_scalar_add(wfg[:], psg[g][:], CT["brp"][0:32, 0:1])
            wfg16 = wp.tile([32, 512], F16, tag=f"wfg16{g}", name=f"wfg16{g}")
            nc.scalar.copy(wfg16[:], wfg[:])
            nc.sync.dma_start(wf_d[:, jj * 512:(jj + 1) * 512]
                              .rearrange("c s -> c s"), wfg[:])
            nc.sync.dma_start(
                wf16_d[:, 1 + 2 * jj: 3 + 2 * jj, :].rearrange("c r w -> c (r w)"),
                wfg16[:])
    for c in range(CH):
        nc.sync.dma_start(wf16_d[c, 0, :], wf16_d[c, 1, :])
        nc.sync.dma_start(wf16_d[c, H + 1, :], wf16_d[c, H, :])


def _image(nc, spool, mpool, ppool, CT, wf_d, wf16_d, clear_d, ci):
    # --- loads ---
    wfr = spool.tile([128, 2 * W], F32R, tag="wfimg")
    for t in range(2):
        nc.sync.dma_start(
            wfr[:, t * W:(t + 1) * W],
            wf_d[ci, t * 128 * W:(t + 1) * 128 * W].rearrange("(p w) -> p w", w=W))
    m1 = mpool.tile([128, 2 * W], F16, tag="m1")
    cc = mpool.tile([128, 2 * W], F16, tag="cc")
    p1 = mpool.tile([128, 2 * W], F16, tag="p1s")
    for t in range(2):
        base = t * 128
        nc.sync.dma_start(m1[:, t * W:(t + 1) * W], wf16_d[ci, base:base + 128, :])
        nc.sync.dma_start(cc[:, t * W:(t + 1) * W], wf16_d[ci, base + 1:base + 129, :])
        nc.sync.dma_start(p1[:, t * W:(t + 1) * W], wf16_d[ci, base + 2:base + 130, :])
    r255 = mpool.tile([1, W], F16, tag="r255")
    nc.sync.dma_start(r255[:], wf16_d[ci, H, :].rearrange("(a w) -> a w", a=1))

    # --- vertical clamped sort3 -> lo/me/hi ---
    lo = mpool.tile([128, 2 * W], F16, tag="lo")
    me = mpool.tile([128, 2 * W], F16, tag="me")
    hi = mpool.tile([128, 2 * W], F16, tag="hi")
    tv = mpool.tile([128, 2 * W], F16, tag="tv")
    mn = mpool.tile([128, 2 * W], F16, tag="mnv")
    mx = mpool.tile([128, 2 * W], F16, tag="mxv")
    nc.vector.tensor_tensor(mn[:], cc[:], p1[:], AT.min)
    nc.vector.tensor_tensor(mx[:], cc[:], p1[:], AT.max)
    nc.vector.tensor_tensor(lo[:], m1[:], mn[:], AT.min)
    nc.vector.tensor_tensor(hi[:], m1[:], mx[:], AT.max)
    nc.vector.tensor_tensor(tv[:], m1[:], mx[:], AT.min)
    nc.vector.tensor_tensor(me[:], tv[:], mn[:], AT.max)

    # --- horizontal stage -> Mx ---
    Ax = mpool.tile([128, 2 * W], F16, tag="Ax")
    Cx = mpool.tile([128, 2 * W], F16, tag="Cx")
    Bx = mpool.tile([128, 2 * W], F16, tag="Bx")
    Mx = mpool.tile([128, 2 * W], F16, tag="Mx")
    h1 = mpool.tile([128, 2 * W], F16, tag="h1")
    h2 = mpool.tile([128, 2 * W], F16, tag="h2")
    for t in range(2):
        o = t * W
        nc.vector.tensor_tensor(h1[:, o:o + W - 1], lo[:, o:o + W - 1], lo[:, o + 1:o + W], AT.max)
        nc.vector.tensor_tensor(Ax[:, o + 1:o + W - 1], h1[:, o:o + W - 2], lo[:, o + 2:o + W], AT.max)
        nc.vector.tensor_copy(Ax[:, o:o + 1], h1[:, o:o + 1])
        nc.vector.tensor_copy(Ax[:, o + W - 1:o + W], h1[:, o + W - 2:o + W - 1])
        nc.vector.tensor_tensor(h2[:, o:o + W - 1], hi[:, o:o + W - 1], hi[:, o + 1:o + W], AT.min)
        nc.vector.tensor_tensor(Cx[:, o + 1:o + W - 1], h2[:, o:o + W - 2], hi[:, o + 2:o + W], AT.min)
        nc.vector.tensor_copy(Cx[:, o:o + 1], h2[:, o:o + 1])
        nc.vector.tensor_copy(Cx[:, o + W - 1:o + W], h2[:, o + W - 2:o + W - 1])
        nc.vector.tensor_tensor(h1[:, o:o + W - 1], me[:, o:o + W - 1], me[:, o + 1:o + W], AT.min)
        nc.vector.tensor_tensor(h2[:, o:o + W - 1], me[:, o:o + W - 1], me[:, o + 1:o + W], AT.max)
        nc.vector.tensor_tensor(Bx[:, o + 1:o + W - 1], me[:, o:o + W - 2], h2[:, o + 1:o + W - 1], AT.min)
        nc.vector.tensor_tensor(Bx[:, o + 1:o + W - 1], Bx[:, o + 1:o + W - 1], h1[:, o + 1:o + W - 1], AT.max)
        nc.vector.tensor_copy(Bx[:, o:o + 1], me[:, o:o + 1])
        nc.vector.tensor_copy(Bx[:, o + W - 1:o + W], me[:, o + W - 1:o + W])
        nc.vector.tensor_tensor(h1[:, o:o + W], Ax[:, o:o + W], Bx[:, o:o + W], AT.min)
        nc.vector.tensor_tensor(h2[:, o:o + W], Ax[:, o:o + W], Bx[:, o:o + W], AT.max)
        nc.vector.tensor_tensor(h2[:, o:o + W], h2[:, o:o + W], Cx[:, o:o + W], AT.min)
        nc.vector.tensor_tensor(Mx[:, o:o + W], h2[:, o:o + W], h1[:, o:o + W], AT.max)

    # --- D stats ---
    Dc = mpool.tile([128, 2 * W], F16, tag="Dc")
    D2 = mpool.tile([128, 2 * W], F32, tag="D2")
    nc.vector.tensor_sub(Dc[:], Mx[:], cc[:])
    nc.vector.tensor_mul(D2[:], Dc[:], Dc[:])
    ST = spool.tile([128, 8], F32R, tag="ST")
    red = spool.tile([128, 8], F32, tag="red")
    nc.vector.reduce_sum(red[:, 0:2], Dc.rearrange("p (t w) -> p t w", t=2), axis=AX.X)
    nc.vector.reduce_sum(red[:, 2:4], D2.rearrange("p (t w) -> p t w", t=2), axis=AX.X)
    nc.vector.tensor_add(ST[:, 0:1], red[:, 0:1], red[:, 1:2])
    nc.vector.tensor_add(ST[:, 1:2], red[:, 2:3], red[:, 3:4])
    # left/right vertical-median strips (weight 19)
    sl_ = spool.tile([128, 12], F32, tag="strips")
    nc.vector.tensor_sub(sl_[:, 0:1], me[:, 0:1], cc[:, 0:1])
    nc.vector.tensor_sub(sl_[:, 1:2], me[:, W - 1:W], cc[:, W - 1:W])
    nc.vector.tensor_sub(sl_[:, 2:3], me[:, W:W + 1], cc[:, W:W + 1])
    nc.vector.tensor_sub(sl_[:, 3:4], me[:, 2 * W - 1:2 * W], cc[:, 2 * W - 1:2 * W])
    for q in range(4):
        nc.vector.tensor_mul(sl_[:, 4 + q:5 + q], sl_[:, q:q + 1], sl_[:, q:q + 1])
    nc.vector.tensor_add(sl_[:, 8:9], sl_[:, 0:1], sl_[:, 1:2])
    nc.vector.tensor_add(sl_[:, 9:10], sl_[:, 2:3], sl_[:, 3:4])
    nc.vector.tensor_add(sl_[:, 8:9], sl_[:, 8:9], sl_[:, 9:10])
    nc.vector.tensor_add(sl_[:, 10:11], sl_[:, 4:5], sl_[:, 5:6])
    nc.vector.tensor_add(sl_[:, 11:12], sl_[:, 6:7], sl_[:, 7:8])
    nc.vector.tensor_add(sl_[:, 10:11], sl_[:, 10:11], sl_[:, 11:12])
    nc.vector.tensor_scalar_mul(ST[:, 2:3], sl_[:, 8:9], 19.0)
    nc.vector.tensor_scalar_mul(ST[:, 3:4], sl_[:, 10:11], 19.0)
    # zero ST[0:1, 4:6] then accumulate the two 1-D strips (x19 inside)
    nc.vector.tensor_scalar_mul(ST[0:1, 4:6], CT["onesr"][0:1, 0:2], 0.0)
    _strip1d(nc, spool, ST, cc[0:1, 0:W])
    _strip1d(nc, spool, ST, r255[0:1, 0:W])

    _nsr.ci = ci
    nsr128 = _nsr(nc, spool, mpool, ppool, CT, ST, cc)

    # --- S1: Ut[c,k] = sum_r wf[r,c] At[r,k] ---
    ps_ut = ppool.tile([128, 2 * KH], F32, tag="ps_ut")
    ps_uti = ppool.tile([128, 2 * KH], F32, tag="ps_uti")
    for mt in range(2):
        for kt in range(2):
            lhs = wfr[:, kt * W + mt * 128: kt * W + mt * 128 + 128]
            nc.tensor.matmul(ps_ut[:, mt * KH:(mt + 1) * KH], lhs,
                             CT["AtR"][:, kt * N: kt * N + KH],
                             start=(kt == 0), stop=(kt == 1))
            nc.tensor.matmul(ps_uti[:, mt * KH:(mt + 1) * KH], lhs,
                             CT["AtI"][:, kt * N: kt * N + KH],
                             start=(kt == 0), stop=(kt == 1))
    UtR = spool.tile([128, 2 * KH], F32R, tag="UtR")
    UtI = spool.tile([128, 2 * KH], F32R, tag="UtI")
    nc.vector.tensor_copy(UtR[:], ps_ut[:])
    nc.scalar.copy(UtI[:], ps_uti[:])

    # --- S2: Gt[l,k] = sum_c At[c,l] Ut[c,k] (complex) ---
    ps_gr = ppool.tile([128, 3 * KH], F32, tag="ps_gr")
    ps_gi = ppool.tile([128, 3 * KH], F32, tag="ps_gi")
    for lt in range(3):
        rs = LT[lt]
        o = slice(lt * KH, lt * KH + KH)
        for kt in range(2):
            lA = slice(kt * N + lt * 128, kt * N + lt * 128 + rs)
            uR = UtR[:, kt * KH:(kt + 1) * KH]
            uI = UtI[:, kt * KH:(kt + 1) * KH]
            nc.tensor.matmul(ps_gr[0:rs, o], CT["AtR"][:, lA], uR, start=(kt == 0), stop=False)
            nc.tensor.matmul(ps_gi[0:rs, o], CT["AtR"][:, lA], uI, start=(kt == 0), stop=False)
        for kt in range(2):
            lA = slice(kt * N + lt * 128, kt * N + lt * 128 + rs)
            uR = UtR[:, kt * KH:(kt + 1) * KH]
            uI = UtI[:, kt * KH:(kt + 1) * KH]
            nc.tensor.matmul(ps_gr[0:rs, o], CT["AtIn"][:, lA], uI, start=False, stop=(kt == 1))
            nc.tensor.matmul(ps_gi[0:rs, o], CT["AtI"][:, lA], uR, start=False, stop=(kt == 1))

    # --- Wiener elementwise -> Zt (f32r) ---
    ZtR = spool.tile([128, 3 * KH], F32R, tag="ZtR")
    ZtI = spool.tile([128, 3 * KH], F32R, tag="ZtI")
    Rv = spool.tile([128, 3 * KH], F32, tag="Rv")
    t1 = spool.tile([128, KH], F32, tag="wt1")
    t2 = spool.tile([128, KH], F32, tag="wt2")
    for lt in range(3):
        rs = LT[lt]
        o = slice(lt * KH, lt * KH + KH)
        nc.vector.tensor_scalar_add(Rv[0:rs, o], CT["Dt"][0:rs, o], nsr128[0:rs, 0:1])
        nc.vector.reciprocal(Rv[0:rs, o], Rv[0:rs, o])
        nc.vector.tensor_mul(t1[0:rs, :], ps_gr[0:rs, o], CT["otfR"][0:rs, o])
        nc.vector.tensor_mul(t2[0:rs, :], ps_gi[0:rs, o], CT["otfI"][0:rs, o])
        nc.vector.tensor_add(t1[0:rs, :], t1[0:rs, :], t2[0:rs, :])
        nc.vector.tensor_mul(ZtR[0:rs, o], t1[0:rs, :], Rv[0:rs, o])
        nc.vector.tensor_mul(t1[0:rs, :], ps_gi[0:rs, o], CT["otfR"][0:rs, o])
        nc.vector.tensor_mul(t2[0:rs, :], ps_gr[0:rs, o], CT["otfI"][0:rs, o])
        nc.vector.tensor_sub(t1[0:rs, :], t1[0:rs, :], t2[0:rs, :])
        nc.vector.tensor_mul(ZtI[0:rs, o], t1[0:rs, :], Rv[0:rs, o])

    # --- S3: V[k,j] = sum_l Zt[l,k] B1t[l,j] (complex) ---
    ps_vr = ppool.tile([128, 2 * W], F32, tag="ps_vr")
    ps_vi = ppool.tile([128, 2 * W], F32, tag="ps_vi")
    for mt in range(2):
        ms = MT[mt]
        o = slice(mt * W, mt * W + W)
        for lt in range(3):
            rs = LT[lt]
            zR = ZtR[0:rs, lt * KH + mt * 128: lt * KH + mt * 128 + ms]
            zI = ZtI[0:rs, lt * KH + mt * 128: lt * KH + mt * 128 + ms]
            bR = CT["B1tR"][0:rs, lt * H:(lt + 1) * H]
            bI = CT["B1tI"][0:rs, lt * H:(lt + 1) * H]
            bIn = CT["B1tIn"][0:rs, lt * H:(lt + 1) * H]
            nc.tensor.matmul(ps_vr[0:ms, o], zR, bR, start=(lt == 0), stop=False)
            nc.tensor.matmul(ps_vi[0:ms, o], zR, bI, start=(lt == 0), stop=False)
            nc.tensor.matmul(ps_vr[0:ms, o], zI, bIn, start=False, stop=(lt == 2))
            nc.tensor.matmul(ps_vi[0:ms, o], zI, bR, start=False, stop=(lt == 2))
    VR = spool.tile([128, 2 * W], F32R, tag="VR")
    VI = spool.tile([128, 2 * W], F32R, tag="VI")
    nc.vector.tensor_copy(VR[:, 0:W], ps_vr[:, 0:W])
    nc.scalar.copy(VI[:, 0:W], ps_vi[:, 0:W])
    nc.vector.tensor_copy(VR[0:MT[1], W:2 * W], ps_vr[0:MT[1], W:2 * W])
    nc.scalar.copy(VI[0:MT[1], W:2 * W], ps_vi[0:MT[1], W:2 * W])

    # --- S4: img[i,j] = sum_k B4[i,k] VR[k,j] - B4i[i,k] VI[k,j] ---
    ps_o = ppool.tile([128, 2 * W], F32, tag="ps_img")
    ps_oa = ps_o[:, 0:W]
    ps_ob = ps_o[:, W:2 * W]
    for mt, ps in ((0, ps_oa), (1, ps_ob)):
        for kt in range(2):
            ks = MT[kt]
            b4r = CT["B4R"][0:ks, kt * H + mt * 128: kt * H + mt * 128 + 128]
            b4i = CT["B4In"][0:ks, kt * H + mt * 128: kt * H + mt * 128 + 128]
            nc.tensor.matmul(ps[:], b4r, VR[0:ks, kt * W:(kt + 1) * W],
                             start=(kt == 0), stop=False)
            nc.tensor.matmul(ps[:], b4i, VI[0:ks, kt * W:(kt + 1) * W],
                             start=False, stop=(kt == 1))
    img = spool.tile([128, 2 * W], F32R, tag="img")
    nc.vector.tensor_copy(img[:, 0:W], ps_oa)
    nc.scalar.copy(img[:, W:2 * W], ps_ob)
    for t in range(2):
        nc.sync.dma_start(
            clear_d[ci, t * 128 * W:(t + 1) * 128 * W].rearrange("(p w) -> p w", w=W),
            img[:, t * W:(t + 1) * W])


def _strip1d(nc, spool, ST, row):
    """Accumulate 19*(sum, sumsq) of (clamped-1D-med3(row) - row) into
    ST[0:1, 4:6]. row: [1, W] fp16."""
    h = spool.tile([1, 4 * W], F32, tag="strip1d")
    mnp = h[:, 0:W - 1]
    mxp = h[:, W:2 * W - 1]
    med = h[:, 2 * W:3 * W]
    nc.vector.tensor_tensor(mnp[:], row[:, 0:W - 1], row[:, 1:W], AT.min)
    nc.vector.tensor_tensor(mxp[:], row[:, 0:W - 1], row[:, 1:W], AT.max)
    nc.vector.tensor_tensor(h[:, 3 * W + 1:4 * W - 1], row[:, 0:W - 2],
                            mxp[:, 1:W - 1], AT.min)
    nc.vector.tensor_tensor(med[:, 1:W - 1], h[:, 3 * W + 1:4 * W - 1],
                            mnp[:, 1:W - 1], AT.max)
    nc.vector.tensor_copy(med[:, 0:1], row[:, 0:1])
    nc.vector.tensor_copy(med[:, W - 1:W], row[:, W - 1:W])
    d = h[:, 3 * W:4 * W]
    nc.vector.tensor_sub(d[:], med[:], row[:])
    s = spool.tile([1, 2], F32, tag="strip1d_s")
    nc.vector.reduce_sum(s[:, 0:1], d[:], axis=AX.X)
    d2 = h[:, 0:W]
    nc.vector.tensor_mul(d2[:], d[:], d[:])
    nc.vector.reduce_sum(s[:, 1:2], d2[:], axis=AX.X)
    nc.vector.tensor_scalar_mul(s[:], s[:], 19.0)
    nc.vector.tensor_add(ST[0:1, 4:6], ST[0:1, 4:6], s[:, 0:2])


def _nsr(nc, spool, mpool, ppool, CT, ST, cc):
    u16 = CT["u16"]
    wsq = mpool.tile([128, 2 * W], F16, tag="wsq")
    nc.vector.tensor_mul(wsq[:], cc[:], cc[:])
    # u-weighted rowsums: rowsum[c] for c = m*128+p; cols (2m, 2m+1) = (W, W2)
    ps_u = ppool.tile([128, 8], F32, tag="ps_small")
    for m in range(2):
        for t in range(2):
            nc.tensor.matmul(ps_u[:, 2 * m:2 * m + 1],
                             cc[:, t * W + m * 128: t * W + m * 128 + 128],
                             u16[:, t:t + 1], start=(t == 0), stop=(t == 1))
            nc.tensor.matmul(ps_u[:, 2 * m + 1:2 * m + 2],
                             wsq[:, t * W + m * 128: t * W + m * 128 + 128],
                             u16[:, t:t + 1], start=(t == 0), stop=(t == 1))
    rsum = spool.tile([128, 4], F16, tag="rsum16")
    nc.vector.tensor_copy(rsum[:], ps_u[:, 0:4])
    # column weights: m=0 cols weighted by u16[:,0] (20 at c=0), m=1 by u16[:,1]
    ps_t = ppool.tile([4, 8], F32, tag="ps_small")
    nc.tensor.matmul(ps_t[0:1, 0:2], u16[:, 0:1], rsum[:, 0:2], start=True, stop=True)
    nc.tensor.matmul(ps_t[0:1, 2:4], u16[:, 1:2], rsum[:, 2:4], start=True, stop=True)
    # D-stat cross-partition sums
    nc.tensor.matmul(ps_t[0:1, 4:8], CT["onesc"][:], ST[:, 0:4], start=True, stop=True)
    sW = spool.tile([1, 16], F32, tag="scal")
    nc.vector.tensor_copy(sW[:, 0:8], ps_t[0:1, 0:8])
    # sums: u-wf = c0+c2 ; u-wf2 = c1+c3 ; sD = c4+c6+ST[0,4] ; sD2 = c5+c7+ST[0,5]
    nc.vector.tensor_add(sW[:, 8:9], sW[:, 0:1], sW[:, 2:3])
    nc.vector.tensor_add(sW[:, 9:10], sW[:, 1:2], sW[:, 3:4])
    nc.vector.tensor_add(sW[:, 10:11], sW[:, 4:5], sW[:, 6:7])
    nc.vector.tensor_add(sW[:, 10:11], sW[:, 10:11], ST[0:1, 4:5])
    nc.vector.tensor_add(sW[:, 11:12], sW[:, 5:6], sW[:, 7:8])
    nc.vector.tensor_add(sW[:, 11:12], sW[:, 11:12], ST[0:1, 5:6])
    ninv, nm1inv = 1.0 / NTOT, 1.0 / (NTOT - 1)
    sc = spool.tile([1, 8], F32, tag="scal2")
    # var_n = (sD2 - sD^2/n)/(n-1)
    nc.vector.tensor_mul(sc[:, 0:1], sW[:, 10:11], sW[:, 10:11])
    nc.vector.tensor_scalar_mul(sc[:, 0:1], sc[:, 0:1], ninv)
    nc.vector.tensor_sub(sc[:, 0:1], sW[:, 11:12], sc[:, 0:1])
    nc.vector.tensor_scalar_mul(sc[:, 0:1], sc[:, 0:1], nm1inv)
    # var_s
    nc.vector.tensor_mul(sc[:, 1:2], sW[:, 8:9], sW[:, 8:9])
    nc.vector.tensor_scalar_mul(sc[:, 1:2], sc[:, 1:2], ninv)
    nc.vector.tensor_sub(sc[:, 1:2], sW[:, 9:10], sc[:, 1:2])
    nc.vector.tensor_scalar_mul(sc[:, 1:2], sc[:, 1:2], nm1inv)
    # rsqrt(var_s) via recip seed + NR
    nc.vector.reciprocal(sc[:, 2:3], sc[:, 1:2])
    for _ in range(4):
        nc.vector.tensor_mul(sc[:, 3:4], sc[:, 2:3], sc[:, 2:3])
        nc.vector.tensor_mul(sc[:, 3:4], sc[:, 3:4], sc[:, 1:2])
        nc.vector.tensor_scalar(sc[:, 3:4], sc[:, 3:4], -0.5, 1.5, AT.mult, AT.add)
        nc.vector.tensor_mul(sc[:, 2:3], sc[:, 2:3], sc[:, 3:4])
    nc.vector.tensor_mul(sc[:, 4:5], sc[:, 0:1], sc[:, 2:3])
    nc.vector.tensor_scalar_mul(sc[:, 4:5], sc[:, 4:5], 8.0 / 30.0)
    nsr_r = spool.tile([1, 1], F32, tag="nsr_r")
    nc.vector.tensor_copy(nsr_r[:], sc[:, 4:5])
    ps_b = ppool.tile([128, 8], F32, tag="ps_small")
    nc.tensor.matmul(ps_b[:, 0:1], CT["onesr"][:], nsr_r[:], start=True, stop=True)
    nsr128 = spool.tile([128, 1], F32, tag="nsr128")
    nc.vector.tensor_copy(nsr128[:], ps_b[:, 0:1])
    nc.sync.dma_start(nc._dbg_nsr[_nsr.ci, :].rearrange("(a b) -> a b", a=1), nsr128[0:1, 0:1])
    return nsr128


def kernel(x, kernel, w_reduce, b_reduce, w_expand, b_expand):
    if "nc" not in _CACHE:
        _CACHE["nc"] = _build()
        _CACHE["consts"] = _host_constants()
    nc = _CACHE["nc"]
    consts = _CACHE["consts"]

    x = np.ascontiguousarray(x, np.float32)
    kern = np.ascontiguousarray(kernel, np.float32)
    xb_rounded = [_round_f32r(x[b].reshape(NF, NPIX)) for b in range(B)]
    in_maps = []
    for core in range(8):
        b, h = core // 2, core % 2
        m = dict(consts)
        m["x"] = xb_rounded[b]
        m["wrT"] = _round_f32r(np.ascontiguousarray(w_reduce[CH * h: CH * h + CH, :].T))
        m["weT"] = _round_f32r(np.ascontiguousarray(w_expand[:, CH * h: CH * h + CH].T))
        m["kern"] = _round_f32r(kern[b, 0])
        m["brp"] = np.tile(np.asarray(b_reduce[CH * h:CH * h + CH], np.float32), 4)[:, None].copy()
        in_maps.append(m)
    res = run_bass_kernel_spmd(nc, in_maps, list(range(8)), trace=False)
    out = np.zeros((B, NF, H, W), np.float32)
    for core in range(8):
        out[core // 2] += res.results[core]["pout"].reshape(NF, H, W)
    out += np.asarray(b_expand, np.float32)[None, :, None, None]
    return out.astype(np.float32)


if __name__ == "__main__":
    import reference
    inputs = reference.setup_inputs()
    out = kernel(**{k: np.asarray(v) for k, v in inputs.items()})
    print("out", out.shape, out.dtype)
